# revision 20
# baseline (speedup 1.0000x reference)
"""Trainium2 Bass kernel for nn_MultiHeadSelfAttention_49160195670596.

Strategy: tensor-parallel over the 8 heads (one head per NeuronCore).
The reference's torch-style .view from (H*B, L, D) to (B, L, H*D) maps
output batch b' to head h = b'//2, so each core computes its two output
batches fully locally -- no collectives.

Per core (head h) the math is restructured to minimize engine load:
  A_h     = W_k_h^T @ W_q_h / sqrt(D)          (host, fp64 -> bf16)
  t_T     = A_h^T @ x_T                        (one matmul replaces both
                                                Q and K projections)
  s_T[k,q]= t_T[:,k-tile]^T @ x_T              (scores transposed: softmax
                                                axis=q is the free axis)
  er      = exp(s_T)            (ScalarE, bf16 out, straight from PSUM)
  e       = er * keep_T, S[k] += row-sum       (scalar_tensor_tensor with
                                                accum_out; all-SBUF bf16
                                                operands hit the DVE 4x
                                                mode; split DVE/GpSimd)
  xs_i    = xN * (1/S[k])                      (normalizer folded into x)
  g_T[d,q]+= xs_i^T @ e_i                      (W_v folded into the final
                                                projection weights)
  out_T   = sum_j wf_j^T @ g-scramble + b_o    (wf = wvT_h @ woT_j)

All matmul SBUF operands are bf16 (PE speed identical to f32r, fp32 PSUM
accumulate).  x, x-natural and the keep-mask are loaded into SBUF once as
consts; the per-iteration loop does pure compute + one output DMA.
Biases are zero in this problem's setup_inputs(); a numpy fallback covers
the general case.
"""
import math
import numpy as np
import ml_dtypes

import concourse.bass as bass
import concourse.tile as tile
from concourse import bacc, mybir
from concourse.bass import ts
from concourse.bass_utils import run_bass_kernel_spmd

B, L, D, H = 16, 512, 128, 8
NCORES = 8
KT = L // 128  # 4 k-tiles per batch
LAG = 3        # software-pipeline depth (batches) between scores and AV
TLEAD = 1      # t-projection runs this many batches ahead of its scores
UNROLL = 4     # logical kernel executions per hardware loop iteration
ABLATE = None  # None | "stt" (timing-only: skip mask+rowsum pass)

f32 = mybir.dt.float32
bf16 = mybir.dt.bfloat16

_CACHE = {}

ZERO_BIAS = True  # kept for test.py compat; only True is supported on HW


def _build(reps=1):
    nc = bacc.Bacc()
    xT_d = nc.dram_tensor("xT", [D, B, L], bf16, kind="ExternalInput")
    xN_d = nc.dram_tensor("xN", [128, B, KT, D], bf16, kind="ExternalInput")
    mk_d = nc.dram_tensor("keep", [128, B, KT, L], bf16, kind="ExternalInput")
    A_d = nc.dram_tensor("Ah", [D, D], bf16, kind="ExternalInput")
    wf_d = nc.dram_tensor("wf", [H * D, D], bf16, kind="ExternalInput")
    bo_d = nc.dram_tensor("bo", [D, 1], f32, kind="ExternalInput")
    out_d = nc.dram_tensor("out", [D, 2 * L], f32, kind="ExternalOutput")

    with tile.TileContext(nc) as tc:
        with (
            tc.tile_pool(name="const", bufs=1) as const,
            tc.tile_pool(name="tts", bufs=4) as tts,
            tc.tile_pool(name="ers", bufs=(8 if ABLATE == "stt" else 4)) as ers,
            tc.tile_pool(name="es", bufs=16) as es,
            tc.tile_pool(name="xss", bufs=8) as xss,
            tc.tile_pool(name="sts", bufs=10) as sts,
            tc.tile_pool(name="attst", bufs=1) as attst,
            tc.tile_pool(name="outs", bufs=1) as outs,
            tc.tile_pool(name="ps_sc", bufs=2, space="PSUM") as ps_sc,
            tc.tile_pool(name="ps_t", bufs=2, space="PSUM") as ps_t,
            tc.tile_pool(name="ps_g", bufs=1, space="PSUM") as ps_g,
            tc.tile_pool(name="ps_o", bufs=1, space="PSUM") as ps_o,
        ):
            import contextlib

            # ---- consts: everything input-side lives in SBUF ----
            xT = const.tile([D, B, L], bf16)
            nc.sync.dma_start(xT, xT_d[:, :, :])
            xN = const.tile([128, B, KT, D], bf16)
            nc.sync.dma_start(xN, xN_d[:, :, :, :])
            mk = const.tile([128, B, KT, L], bf16)
            nc.sync.dma_start(mk, mk_d[:, :, :, :])
            A = const.tile([D, D], bf16)
            nc.sync.dma_start(A, A_d[:, :])
            # wf [1024,128] -> SBUF [e=128, j=8, d'=128]
            wf = const.tile([D, H, D], bf16)
            nc.sync.dma_start(wf, wf_d[:, :].rearrange("(j e) d -> e j d", j=H))
            bo = const.tile([D, 1], f32)
            nc.sync.dma_start(bo, bo_d[:, :])
            att0 = attst.tile([D, B * L // 2], bf16)
            att1 = attst.tile([D, B * L // 2], bf16)
            att_stores = (att0, att1)

            v = dict(
                xT=xT, xN=xN, mk=mk, A=A, wf=wf, bo=bo,
                att_stores=att_stores, out_d=out_d,
                tts=tts, ers=ers, es=es, xss=xss, sts=sts, outs=outs,
                ps_sc=ps_sc, ps_t=ps_t, ps_g=ps_g, ps_o=ps_o,
            )
            # Unroll UNROLL bodies per hardware loop iteration so the For_i
            # all-engine barrier (+ pipeline ramp/drain) amortizes; the
            # software pipeline runs continuously across the unrolled
            # bodies.  reps = UNROLL*(reps//UNROLL) + reps%UNROLL exactly.
            n_loop, n_extra = divmod(reps, UNROLL)
            if n_loop > 1:
                with tc.For_i(0, n_loop, 1, hint_engines=(
                        mybir.EngineType.PE, mybir.EngineType.DVE,
                        mybir.EngineType.Activation, mybir.EngineType.SP,
                        mybir.EngineType.Pool)):
                    _emit_body(nc, tc, v, nbody=UNROLL)
            elif n_loop == 1:
                _emit_body(nc, tc, v, nbody=UNROLL)
            if n_extra:
                _emit_body(nc, tc, v, nbody=n_extra)
    nc.compile()
    return nc


def _emit_body(nc, tc, v, nbody=1):
    """Emit `nbody` back-to-back logical kernel executions as one
    continuously-pipelined batch stream (no pipeline drain between them)."""
    xT, xN, mk, A, wf, bo = v["xT"], v["xN"], v["mk"], v["A"], v["wf"], v["bo"]
    att_stores, out_d = v["att_stores"], v["out_d"]
    tts, ers, es, xss, sts, outs = (
        v["tts"], v["ers"], v["es"], v["xss"], v["sts"], v["outs"])
    ps_sc, ps_t, ps_g, ps_o = v["ps_sc"], v["ps_t"], v["ps_g"], v["ps_o"]

    NU = nbody * B
    # per-unit state carried across the software pipeline (keyed by u)
    tT_tiles = {}
    e_tiles = {}
    r_tiles = {}

    def emit_t(u):
        """t-projection (replaces Q and K projections)."""
        b = u % B
        t_ps = ps_t.tile([D, L], f32, tag="t")
        nc.tensor.matmul(t_ps, A, xT[:, b, :], start=True, stop=True)
        tT = tts.tile([D, L], bf16)
        nc.scalar.copy(tT, t_ps)
        tT_tiles[u] = tT

    def emit_front(u):
        """scores, exp, mask+rowsum, reciprocal."""
        b = u % B
        xT_b = xT[:, b, :]
        tT = tT_tiles.pop(u)
        S = sts.tile([128, KT], f32, tag="S")
        r = sts.tile([128, KT], f32, tag="r")
        e_list = []
        sc_pair = {}
        for i in range(KT):
            if i % 2 == 0:
                sc2 = ps_sc.tile([128, 2, L], f32, tag="sc")
                er2 = ers.tile([128, 2, L], bf16)
                sc_pair[i] = (sc2, er2)
            sc2, er2 = sc_pair[i - i % 2]
            nc.tensor.matmul(
                sc2[:, i % 2, :], tT[:, ts(i, 128)], xT_b,
                start=True, stop=True,
            )
            if i % 2 == 1:
                nc.scalar.activation(er2, sc2, mybir.ActivationFunctionType.Exp)
                for ii in (i - 1, i):
                    if ABLATE == "stt":
                        e_list.append(er2[:, ii % 2, :])
                        continue
                    e = es.tile([128, L], bf16)
                    nc.vector.scalar_tensor_tensor(
                        out=e, in0=er2[:, ii % 2, :], scalar=1.0,
                        in1=mk[:, b, ii, :],
                        op0=mybir.AluOpType.bypass,
                        op1=mybir.AluOpType.mult,
                        accum_out=S[:, ii : ii + 1],
                    )
                    e_list.append(e)
        if ABLATE == "stt":
            nc.vector.memset(r, 1.0)
        else:
            nc.vector.reciprocal(r, S)
        e_tiles[u] = e_list
        r_tiles[u] = r

    def emit_back(u):
        """normalizer-scaled x, AV matmul, att-store evac."""
        b = u % B
        r = r_tiles.pop(u)
        es_u = e_tiles.pop(u)
        g_ps = ps_g.tile([D, L], f32, tag="g")
        for i in range(KT):
            xs_i = xss.tile([128, D], bf16)
            nc.vector.tensor_scalar_mul(xs_i, xN[:, b, i, :], r[:, i : i + 1])
            nc.tensor.matmul(
                g_ps, xs_i, es_u[i], start=(i == 0), stop=(i == KT - 1)
            )
        dst = att_stores[(b // 8)][:, ts(b % 8, L)]
        if b % 2 == 0:
            nc.scalar.copy(dst, g_ps)
        else:
            nc.vector.tensor_copy(dst, g_ps)

    ob = outs.tile([D, 2 * L], f32)

    def emit_final_quarter(u):
        """final projection through the torch-view scramble for the
        256-column chunk of out_T fed by batches u-3..u:
        out_T[d', m] = sum_j wf_j.T @ att_store[:, 8*m + j]"""
        b = u % B
        half = b // 8
        chunk = (b // 4) % 2
        RH = att_stores[half].rearrange("p (m j) -> p m j", j=H)
        o_ps = ps_o.tile([D, L], f32, tag="o")
        osl = o_ps[:, ts(chunk, 256)]
        for j in range(H):
            nc.tensor.matmul(
                osl, wf[:, j, :], RH[:, chunk * 256 : (chunk + 1) * 256, j],
                start=(j == 0), stop=(j == H - 1),
            )
        csl = slice(half * L + chunk * 256, half * L + chunk * 256 + 256)
        nc.scalar.activation(
            ob[:, csl], osl,
            mybir.ActivationFunctionType.Identity, bias=bo,
        )
        nc.sync.dma_start(out_d[:, csl], ob[:, csl])

    for u in range(NU + LAG):
        if u < TLEAD:
            emit_t(u)
        if u < NU:
            if u + TLEAD < NU:
                emit_t(u + TLEAD)
            emit_front(u)
        if u >= LAG:
            uu = u - LAG
            emit_back(uu)
            if uu % 4 == 3:
                emit_final_quarter(uu)


def _get_nc():
    if "nc" not in _CACHE:
        _CACHE["nc"] = _build()
    return _CACHE["nc"]


def make_in_maps(x, W_q, b_q, W_k, b_k, W_v, b_v, W_o, b_o, pad_mask):
    scale = 1.0 / math.sqrt(D)
    xT = np.ascontiguousarray(x.transpose(2, 0, 1)).astype(ml_dtypes.bfloat16)
    # x natural, tiled so partition p = l within each 128-row k-tile
    xN = np.ascontiguousarray(
        x.reshape(B, KT, 128, D).transpose(2, 0, 1, 3)
    ).astype(ml_dtypes.bfloat16)
    # keep mask, transposed to [k, q] then tiled like xN; bf16 {0,1}
    keepT = (~pad_mask).astype(np.float32).transpose(0, 2, 1)  # [B, L(k), L(q)]
    keep = np.ascontiguousarray(
        keepT.reshape(B, KT, 128, L).transpose(2, 0, 1, 3)
    ).astype(ml_dtypes.bfloat16)
    bo_col = np.ascontiguousarray(b_o[:, None]).astype(np.float32)

    woT64 = W_o.T.astype(np.float64)  # [1024, 128]
    in_maps = []
    for h in range(NCORES):
        sl = slice(h * D, (h + 1) * D)
        A_h = (W_k[sl, :].T.astype(np.float64) @ W_q[sl, :].astype(np.float64)
               ) * scale
        wvT_h = W_v[sl, :].T.astype(np.float64)
        wf = np.concatenate(
            [wvT_h @ woT64[j * 128 : (j + 1) * 128, :] for j in range(H)],
            axis=0,
        )
        in_maps.append({
            "xT": xT,
            "xN": xN,
            "keep": keep,
            "Ah": np.ascontiguousarray(A_h).astype(ml_dtypes.bfloat16),
            "wf": np.ascontiguousarray(wf).astype(ml_dtypes.bfloat16),
            "bo": bo_col,
        })
    return in_maps


def _numpy_reference(x, W_q, b_q, W_k, b_k, W_v, b_v, W_o, b_o, pad_mask):
    x64 = x.astype(np.float64)
    def proj(W, b):
        y = np.einsum("bld,ed->ble", x64, W.astype(np.float64)) + b
        y = y.reshape(B, L, H, D)
        return y.transpose(2, 0, 1, 3).reshape(H * B, L, D)
    q = proj(W_q, b_q)
    k = proj(W_k, b_k)
    vv = proj(W_v, b_v)
    scores = np.einsum("nqd,nkd->nqk", q, k)
    mask = np.tile(pad_mask, (H, 1, 1))
    scores = np.where(mask, -1e9, scores) / math.sqrt(D)
    scores -= scores.max(axis=1, keepdims=True)
    ex = np.exp(scores)
    attn = ex / ex.sum(axis=1, keepdims=True)
    att = np.einsum("nqk,nkd->nqd", attn, vv)
    att = att.reshape(B, L, H * D)
    out = np.einsum("ble,de->bld", att, W_o.astype(np.float64)) + b_o
    return out.astype(np.float32)


def kernel(x, W_q, b_q, W_k, b_k, W_v, b_v, W_o, b_o, pad_mask, **kwargs):
    x = np.asarray(x, dtype=np.float32)
    W_q = np.asarray(W_q, dtype=np.float32)
    W_k = np.asarray(W_k, dtype=np.float32)
    W_v = np.asarray(W_v, dtype=np.float32)
    W_o = np.asarray(W_o, dtype=np.float32)
    b_q = np.asarray(b_q, dtype=np.float32)
    b_k = np.asarray(b_k, dtype=np.float32)
    b_v = np.asarray(b_v, dtype=np.float32)
    b_o = np.asarray(b_o, dtype=np.float32)
    pad_mask = np.asarray(pad_mask).astype(bool)

    if b_q.any() or b_k.any() or b_v.any():
        # general-bias fallback (never hit by this problem's setup_inputs)
        return _numpy_reference(
            x, W_q, b_q, W_k, b_k, W_v, b_v, W_o, b_o, pad_mask)

    in_maps = make_in_maps(x, W_q, b_q, W_k, b_k, W_v, b_v, W_o, b_o, pad_mask)
    nc = _get_nc()
    res = run_bass_kernel_spmd(nc, in_maps, core_ids=list(range(NCORES)))
    # per-core out_T [128, 1024] -> rows 1024h..1024(h+1) of flat [8192, 128]
    flat = np.concatenate([res.results[h]["out"].T for h in range(NCORES)], axis=0)
    return np.ascontiguousarray(flat.reshape(B, L, D), dtype=np.float32)


if __name__ == "__main__":
    rng = np.random.default_rng(0)
    demo = {
        "x": rng.standard_normal((B, L, D), dtype=np.float32),
        "W_q": rng.standard_normal((H * D, D), dtype=np.float32) * 0.04,
        "b_q": np.zeros(H * D, np.float32),
        "W_k": rng.standard_normal((H * D, D), dtype=np.float32) * 0.04,
        "b_k": np.zeros(H * D, np.float32),
        "W_v": rng.standard_normal((H * D, D), dtype=np.float32) * 0.04,
        "b_v": np.zeros(H * D, np.float32),
        "W_o": rng.standard_normal((D, H * D), dtype=np.float32) * 0.04,
        "b_o": np.zeros(D, np.float32),
        "pad_mask": rng.integers(0, 2, (B, L, L)).astype(bool),
    }
    out = kernel(**demo)
    exp = _numpy_reference(**demo)
    err = np.abs(out - exp).max() / np.abs(exp).max()
    print("kernel ran, out shape:", out.shape, "rel err vs numpy:", err)


# revision 24
# speedup vs baseline: 1.3918x; 1.3918x over previous
"""Trainium2 Bass kernel for nn_MultiHeadSelfAttention_49160195670596.

Strategy: tensor-parallel over the 8 heads (one head per NeuronCore).
The reference's torch-style .view from (H*B, L, D) to (B, L, H*D) maps
output batch b' to head h = b'//2, so each core computes its two output
batches fully locally -- no collectives.

Per core (head h) the math is restructured to minimize engine load:
  A_h     = W_k_h^T @ W_q_h / sqrt(D)          (host, fp64 -> bf16)
  t_T     = A_h^T @ x_T                        (one matmul replaces both
                                                Q and K projections)
  s_T[k,q]= t_T[:,k-tile]^T @ x_T              (scores transposed: softmax
                                                axis=q is the free axis)
  er      = exp(s_T)            (ScalarE, bf16 out, straight from PSUM)
  e       = er * keep_T, S[k] += row-sum       (scalar_tensor_tensor with
                                                accum_out; all-SBUF bf16
                                                operands hit the DVE 4x
                                                mode; split DVE/GpSimd)
  xs_i    = xN * (1/S[k])                      (normalizer folded into x)
  g_T[d,q]+= xs_i^T @ e_i                      (W_v folded into the final
                                                projection weights)
  out_T   = sum_j wf_j^T @ g-scramble + b_o    (wf = wvT_h @ woT_j)

All matmul SBUF operands are bf16 (PE speed identical to f32r, fp32 PSUM
accumulate).  x, x-natural and the keep-mask are loaded into SBUF once as
consts; the per-iteration loop does pure compute + one output DMA.
Biases are zero in this problem's setup_inputs(); a numpy fallback covers
the general case.
"""
import math
import numpy as np
import ml_dtypes

import concourse.bass as bass
import concourse.tile as tile
from concourse import bacc, mybir
from concourse.bass import ts
from concourse.bass_utils import run_bass_kernel_spmd

B, L, D, H = 16, 512, 128, 8
NCORES = 8
KT = L // 128  # 4 k-tiles per batch
LAG = 3        # software-pipeline depth (batches) between scores and AV
TLEAD = 1      # t-projection runs this many batches ahead of its scores
UNROLL = 4     # logical kernel executions per hardware loop iteration
ABLATE = None  # None | "stt" | "softmax" (timing-only ablations)

f32 = mybir.dt.float32
bf16 = mybir.dt.bfloat16

_CACHE = {}

ZERO_BIAS = True  # kept for test.py compat; only True is supported on HW


def _build(reps=1):
    nc = bacc.Bacc()
    xT_d = nc.dram_tensor("xT", [D, B, L], bf16, kind="ExternalInput")
    xN_d = nc.dram_tensor("xN", [128, B, KT, D], bf16, kind="ExternalInput")
    mk_d = nc.dram_tensor("keep", [128, B, KT, L], bf16, kind="ExternalInput")
    A_d = nc.dram_tensor("Ah", [D, D], bf16, kind="ExternalInput")
    wf_d = nc.dram_tensor("wf", [H * D, D], bf16, kind="ExternalInput")
    bo_d = nc.dram_tensor("bo", [D, 1], f32, kind="ExternalInput")
    out_d = nc.dram_tensor("out", [D, 2 * L], f32, kind="ExternalOutput")

    with tile.TileContext(nc) as tc:
        with (
            tc.tile_pool(name="const", bufs=1) as const,
            tc.tile_pool(name="tts", bufs=4) as tts,
            tc.tile_pool(name="ers", bufs=(8 if ABLATE == "stt" else 4)) as ers,
            tc.tile_pool(name="es", bufs=16) as es,
            tc.tile_pool(name="xss", bufs=8) as xss,
            tc.tile_pool(name="sts", bufs=10) as sts,
            tc.tile_pool(name="attst", bufs=1) as attst,
            tc.tile_pool(name="outs", bufs=1) as outs,
            tc.tile_pool(name="ps_sc", bufs=2, space="PSUM") as ps_sc,
            tc.tile_pool(name="ps_t", bufs=2, space="PSUM") as ps_t,
            tc.tile_pool(name="ps_g", bufs=1, space="PSUM") as ps_g,
            tc.tile_pool(name="ps_o", bufs=1, space="PSUM") as ps_o,
        ):
            import contextlib

            # ---- consts: everything input-side lives in SBUF ----
            xT = const.tile([D, B, L], bf16)
            nc.sync.dma_start(xT, xT_d[:, :, :])
            xN = const.tile([128, B, KT, D], bf16)
            nc.sync.dma_start(xN, xN_d[:, :, :, :])
            mk = const.tile([128, B, KT, L], bf16)
            nc.sync.dma_start(mk, mk_d[:, :, :, :])
            A = const.tile([D, D], bf16)
            nc.sync.dma_start(A, A_d[:, :])
            # wf [1024,128] -> SBUF [e=128, j=8, d'=128]
            wf = const.tile([D, H, D], bf16)
            nc.sync.dma_start(wf, wf_d[:, :].rearrange("(j e) d -> e j d", j=H))
            bo = const.tile([D, 1], f32)
            nc.sync.dma_start(bo, bo_d[:, :])
            att0 = attst.tile([D, B * L // 2], bf16)
            att1 = attst.tile([D, B * L // 2], bf16)
            att_stores = (att0, att1)

            v = dict(
                xT=xT, xN=xN, mk=mk, A=A, wf=wf, bo=bo,
                att_stores=att_stores, out_d=out_d,
                tts=tts, ers=ers, es=es, xss=xss, sts=sts, outs=outs,
                ps_sc=ps_sc, ps_t=ps_t, ps_g=ps_g, ps_o=ps_o,
            )
            # Unroll UNROLL bodies per hardware loop iteration so the For_i
            # all-engine barrier (+ pipeline ramp/drain) amortizes; the
            # software pipeline runs continuously across the unrolled
            # bodies.  reps = UNROLL*(reps//UNROLL) + reps%UNROLL exactly.
            n_loop, n_extra = divmod(reps, UNROLL)
            if n_loop > 1:
                with tc.For_i(0, n_loop, 1, hint_engines=(
                        mybir.EngineType.PE, mybir.EngineType.DVE,
                        mybir.EngineType.Activation, mybir.EngineType.SP,
                        mybir.EngineType.Pool)):
                    _emit_body(nc, tc, v, nbody=UNROLL)
            elif n_loop == 1:
                _emit_body(nc, tc, v, nbody=UNROLL)
            if n_extra:
                _emit_body(nc, tc, v, nbody=n_extra)
    nc.compile()
    return nc


def _emit_body(nc, tc, v, nbody=1):
    """Emit `nbody` back-to-back logical kernel executions as one
    continuously-pipelined batch stream (no pipeline drain between them)."""
    xT, xN, mk, A, wf, bo = v["xT"], v["xN"], v["mk"], v["A"], v["wf"], v["bo"]
    att_stores, out_d = v["att_stores"], v["out_d"]
    tts, ers, es, xss, sts, outs = (
        v["tts"], v["ers"], v["es"], v["xss"], v["sts"], v["outs"])
    ps_sc, ps_t, ps_g, ps_o = v["ps_sc"], v["ps_t"], v["ps_g"], v["ps_o"]

    NU = nbody * B
    # per-unit state carried across the software pipeline (keyed by u)
    tT_tiles = {}
    e_tiles = {}
    r_tiles = {}

    if ABLATE == "softmax":
        dummy_e = es.tile([128, L], bf16)
        nc.vector.memset(dummy_e, 0.001)
        dummy_r = sts.tile([128, KT], f32, tag="r")
        nc.vector.memset(dummy_r, 1.0)

    def emit_t(u):
        """t-projection (replaces Q and K projections)."""
        b = u % B
        t_ps = ps_t.tile([D, L], f32, tag="t")
        nc.tensor.matmul(t_ps, A, xT[:, b, :], start=True, stop=True)
        tT = tts.tile([D, L], bf16)
        nc.scalar.copy(tT, t_ps)
        tT_tiles[u] = tT

    def emit_front(u):
        """scores, exp, mask+rowsum, reciprocal."""
        b = u % B
        xT_b = xT[:, b, :]
        tT = tT_tiles.pop(u)
        S = sts.tile([128, KT], f32, tag="S")
        r = sts.tile([128, KT], f32, tag="r")
        e_list = []
        sc_pair = {}
        for i in range(KT):
            if i % 2 == 0:
                sc2 = ps_sc.tile([128, 2, L], f32, tag="sc")
                er2 = ers.tile([128, 2, L], bf16)
                sc_pair[i] = (sc2, er2)
            sc2, er2 = sc_pair[i - i % 2]
            nc.tensor.matmul(
                sc2[:, i % 2, :], tT[:, ts(i, 128)], xT_b,
                start=True, stop=True,
            )
            if i % 2 == 1:
                if ABLATE == "softmax":
                    continue
                nc.scalar.activation(er2, sc2, mybir.ActivationFunctionType.Exp)
                for ii in (i - 1, i):
                    if ABLATE == "stt":
                        e_list.append(er2[:, ii % 2, :])
                        continue
                    e = es.tile([128, L], bf16)
                    nc.vector.scalar_tensor_tensor(
                        out=e, in0=er2[:, ii % 2, :], scalar=1.0,
                        in1=mk[:, b, ii, :],
                        op0=mybir.AluOpType.bypass,
                        op1=mybir.AluOpType.mult,
                        accum_out=S[:, ii : ii + 1],
                    )
                    e_list.append(e)
        if ABLATE == "softmax":
            e_tiles[u] = [dummy_e] * KT
            r_tiles[u] = dummy_r
            return
        if ABLATE == "stt":
            nc.vector.memset(r, 1.0)
        else:
            nc.vector.reciprocal(r, S)
        e_tiles[u] = e_list
        r_tiles[u] = r

    def emit_back(u):
        """normalizer-scaled x, AV matmul, att-store evac."""
        b = u % B
        r = r_tiles.pop(u)
        es_u = e_tiles.pop(u)
        g_ps = ps_g.tile([D, L], f32, tag="g")
        for i in range(KT):
            xs_i = xss.tile([128, D], bf16)
            nc.vector.tensor_scalar_mul(xs_i, xN[:, b, i, :], r[:, i : i + 1])
            nc.tensor.matmul(
                g_ps, xs_i, es_u[i], start=(i == 0), stop=(i == KT - 1)
            )
        dst = att_stores[(b // 8)][:, ts(b % 8, L)]
        if b % 2 == 0:
            nc.scalar.copy(dst, g_ps)
        else:
            nc.vector.tensor_copy(dst, g_ps)

    ob = outs.tile([D, 2 * L], f32)

    def emit_final_quarter(u):
        """final projection through the torch-view scramble for the
        256-column chunk of out_T fed by batches u-3..u:
        out_T[d', m] = sum_j wf_j.T @ att_store[:, 8*m + j]"""
        b = u % B
        half = b // 8
        chunk = (b // 4) % 2
        RH = att_stores[half].rearrange("p (m j) -> p m j", j=H)
        o_ps = ps_o.tile([D, L], f32, tag="o")
        osl = o_ps[:, ts(chunk, 256)]
        for j in range(H):
            nc.tensor.matmul(
                osl, wf[:, j, :], RH[:, chunk * 256 : (chunk + 1) * 256, j],
                start=(j == 0), stop=(j == H - 1),
            )
        csl = slice(half * L + chunk * 256, half * L + chunk * 256 + 256)
        nc.scalar.activation(
            ob[:, csl], osl,
            mybir.ActivationFunctionType.Identity, bias=bo,
        )
        nc.sync.dma_start(out_d[:, csl], ob[:, csl])

    for u in range(NU + LAG):
        if u < TLEAD:
            emit_t(u)
        if u < NU:
            if u + TLEAD < NU:
                emit_t(u + TLEAD)
            emit_front(u)
        if u >= LAG:
            uu = u - LAG
            emit_back(uu)
            if uu % 4 == 3:
                emit_final_quarter(uu)


def _get_nc():
    if "nc" not in _CACHE:
        _CACHE["nc"] = _build()
    return _CACHE["nc"]


def make_in_maps(x, W_q, b_q, W_k, b_k, W_v, b_v, W_o, b_o, pad_mask):
    scale = 1.0 / math.sqrt(D)
    xT = np.ascontiguousarray(x.transpose(2, 0, 1)).astype(ml_dtypes.bfloat16)
    # x natural, tiled so partition p = l within each 128-row k-tile
    xN = np.ascontiguousarray(
        x.reshape(B, KT, 128, D).transpose(2, 0, 1, 3)
    ).astype(ml_dtypes.bfloat16)
    # keep mask, transposed to [k, q] then tiled like xN; bf16 {0,1}
    keepT = (~pad_mask).astype(np.float32).transpose(0, 2, 1)  # [B, L(k), L(q)]
    keep = np.ascontiguousarray(
        keepT.reshape(B, KT, 128, L).transpose(2, 0, 1, 3)
    ).astype(ml_dtypes.bfloat16)
    bo_col = np.ascontiguousarray(b_o[:, None]).astype(np.float32)

    woT64 = W_o.T.astype(np.float64)  # [1024, 128]
    in_maps = []
    for h in range(NCORES):
        sl = slice(h * D, (h + 1) * D)
        A_h = (W_k[sl, :].T.astype(np.float64) @ W_q[sl, :].astype(np.float64)
               ) * scale
        wvT_h = W_v[sl, :].T.astype(np.float64)
        wf = np.concatenate(
            [wvT_h @ woT64[j * 128 : (j + 1) * 128, :] for j in range(H)],
            axis=0,
        )
        in_maps.append({
            "xT": xT,
            "xN": xN,
            "keep": keep,
            "Ah": np.ascontiguousarray(A_h).astype(ml_dtypes.bfloat16),
            "wf": np.ascontiguousarray(wf).astype(ml_dtypes.bfloat16),
            "bo": bo_col,
        })
    return in_maps


def _numpy_reference(x, W_q, b_q, W_k, b_k, W_v, b_v, W_o, b_o, pad_mask):
    x64 = x.astype(np.float64)
    def proj(W, b):
        y = np.einsum("bld,ed->ble", x64, W.astype(np.float64)) + b
        y = y.reshape(B, L, H, D)
        return y.transpose(2, 0, 1, 3).reshape(H * B, L, D)
    q = proj(W_q, b_q)
    k = proj(W_k, b_k)
    vv = proj(W_v, b_v)
    scores = np.einsum("nqd,nkd->nqk", q, k)
    mask = np.tile(pad_mask, (H, 1, 1))
    scores = np.where(mask, -1e9, scores) / math.sqrt(D)
    scores -= scores.max(axis=1, keepdims=True)
    ex = np.exp(scores)
    attn = ex / ex.sum(axis=1, keepdims=True)
    att = np.einsum("nqk,nkd->nqd", attn, vv)
    att = att.reshape(B, L, H * D)
    out = np.einsum("ble,de->bld", att, W_o.astype(np.float64)) + b_o
    return out.astype(np.float32)


def kernel(x, W_q, b_q, W_k, b_k, W_v, b_v, W_o, b_o, pad_mask, **kwargs):
    x = np.asarray(x, dtype=np.float32)
    W_q = np.asarray(W_q, dtype=np.float32)
    W_k = np.asarray(W_k, dtype=np.float32)
    W_v = np.asarray(W_v, dtype=np.float32)
    W_o = np.asarray(W_o, dtype=np.float32)
    b_q = np.asarray(b_q, dtype=np.float32)
    b_k = np.asarray(b_k, dtype=np.float32)
    b_v = np.asarray(b_v, dtype=np.float32)
    b_o = np.asarray(b_o, dtype=np.float32)
    pad_mask = np.asarray(pad_mask).astype(bool)

    if b_q.any() or b_k.any() or b_v.any():
        # general-bias fallback (never hit by this problem's setup_inputs)
        return _numpy_reference(
            x, W_q, b_q, W_k, b_k, W_v, b_v, W_o, b_o, pad_mask)

    in_maps = make_in_maps(x, W_q, b_q, W_k, b_k, W_v, b_v, W_o, b_o, pad_mask)
    nc = _get_nc()
    res = run_bass_kernel_spmd(nc, in_maps, core_ids=list(range(NCORES)))
    # per-core out_T [128, 1024] -> rows 1024h..1024(h+1) of flat [8192, 128]
    flat = np.concatenate([res.results[h]["out"].T for h in range(NCORES)], axis=0)
    return np.ascontiguousarray(flat.reshape(B, L, D), dtype=np.float32)


if __name__ == "__main__":
    rng = np.random.default_rng(0)
    demo = {
        "x": rng.standard_normal((B, L, D), dtype=np.float32),
        "W_q": rng.standard_normal((H * D, D), dtype=np.float32) * 0.04,
        "b_q": np.zeros(H * D, np.float32),
        "W_k": rng.standard_normal((H * D, D), dtype=np.float32) * 0.04,
        "b_k": np.zeros(H * D, np.float32),
        "W_v": rng.standard_normal((H * D, D), dtype=np.float32) * 0.04,
        "b_v": np.zeros(H * D, np.float32),
        "W_o": rng.standard_normal((D, H * D), dtype=np.float32) * 0.04,
        "b_o": np.zeros(D, np.float32),
        "pad_mask": rng.integers(0, 2, (B, L, L)).astype(bool),
    }
    out = kernel(**demo)
    exp = _numpy_reference(**demo)
    err = np.abs(out - exp).max() / np.abs(exp).max()
    print("kernel ran, out shape:", out.shape, "rel err vs numpy:", err)
